# revision 37
# baseline (speedup 1.0000x reference)
"""Trainium2 Bass kernel for nn_DirectedEdgeDecoder (gnn_message_passing) — v3.1.

  out[e] = W2 . relu(concat(z1[row_e], z2[col_e]) @ W1 + b1) + b2

Decomposition: per-node projections u1 = z1 @ W1[:D] + b1, u2 = z2 @ W1[D:]
(8 f16 values/node, kernel A), then per-edge combine (kernel B):
r-quantile edge sharding; u1 expanded into a DRAM slot array X by
pseudo-node (deg<=2) broadcast DMAs; u2 expanded into SBUF token streams by
in-quarter col-degree buckets {1,2}; bulk dma_scatter_add routes each token
onto its edge slot; strided readback + relu + .W2 + tree-reduce.

Scheduling (cost-model driven): DMA work balanced across the three
DMA-capable engines (SP / Act / Pool-SWDGE), ordered so quarter q's scatter
dependencies (idx, X-expansion, token stream) complete just in time; the
whole per-slot compute chain runs in f16 with DVE 2x/4x perf modes, with
tail ops placed on Pool/Act to keep DVE the only near-critical engine.
"""
import numpy as np
import concourse.bass as bass
import concourse.mybir as mybir
import concourse.tile as tile
from concourse import bacc
from concourse.bass_utils import run_bass_kernel_spmd

P = 128
N_CORES = 8
N_NODES = 100000
N_EDGES = 800000
D = 128
H = 8

f32 = mybir.dt.float32
f16 = mybir.dt.float16
i16 = mybir.dt.int16

# ------------------------------------------------------------- configuration


class Cfg:
    """Geometry of kernel B. All sizes are compile-time constants."""

    def __init__(self):
        self.NQ = 4                       # slot quarters per core
        self.D1 = 2                       # u1 pseudo-node degree
        self.PN_Q = 13440                 # u1 pseudo-node cap per quarter
        self.TRASH = 256                  # trash slots per quarter
        self.CAP2 = {1: 20224, 2: 3072}   # u2 chunk caps per quarter
        self.NCHUNK = 4                   # scatter instructions per quarter
        self.SQ = self.PN_Q * self.D1 + self.TRASH    # slots / quarter
        assert self.SQ % 256 == 0 and self.SQ < 32768
        self.S = self.NQ * self.SQ
        self.BUCKETS = [1, 2]
        self.TOK2 = {d: c * d for d, c in self.CAP2.items()}
        assert all(v % 128 == 0 for v in self.TOK2.values())
        self.NTOK_USED = sum(self.TOK2.values())      # 26368
        self.KUSED = self.NTOK_USED // 128            # 206
        self.KTOT = 208
        self.NTOKQ = self.KTOT * 128                  # 26624
        self.CHUNK = self.NTOKQ // self.NCHUNK        # 6656 tokens
        assert self.CHUNK % 128 == 0, self.CHUNK
        self.KB = {}
        kb = 0
        for d in self.BUCKETS:
            self.KB[d] = kb
            kb += self.TOK2[d] // 128
        assert kb == self.KUSED
        self.ROWS2_Q = sum(self.CAP2.values())        # U2c rows per quarter
        assert self.TRASH % self.D1 == 0
        self.ROWS1_Q = self.PN_Q + self.TRASH // self.D1
        self.XCOL = 128                   # f16 per X row (256B stride)
        self.IDXW = self.NTOKQ // 16      # idx cols per quarter (1664)
        self.KOUT = self.SQ // 128        # output cols per quarter (212)


CFG = Cfg()

# ---------------------------------------------------------------- kernel A

NC_NODES = N_NODES // N_CORES          # 12500 nodes per core
KN = 98                                # node blocks per core
NP = KN * P                            # 12544 padded nodes per core


def _new_nc():
    return bacc.Bacc(
        "TRN2", target_bir_lowering=False, debug=False, num_devices=N_CORES,
        num_swdge_queues=1,
    )


def build_precompute():
    """Per-core: u[t] = zT[t].T @ W1[t*128:(t+1)*128] (+ b1 if t == 0), f16.

    Inputs : z1T [128, NP] f32 (shard, transposed, padded), z2T likewise,
             W1 [256, 8] f32, b1 [1, 8] f32
    Output : u [2, 128, KN*8] f16 -- u[t][p, k*8+h] = node (k*128+p) proj h
    """
    nc = _new_nc()
    z1T = nc.declare_dram_parameter("z1T", [P, NP], f32, isOutput=False)
    z2T = nc.declare_dram_parameter("z2T", [P, NP], f32, isOutput=False)
    W1 = nc.declare_dram_parameter("W1", [2 * D, H], f32, isOutput=False)
    b1 = nc.declare_dram_parameter("b1", [1, H], f32, isOutput=False)
    u = nc.declare_dram_parameter("u", [2, P, KN * H], f16, isOutput=True)

    CH = 7
    CW = CH * P
    NLOAD = KN // CH                       # 14 loads per table

    # consts go first on Act; z loads start immediately on SP/Pool.
    # 28 loads: SP 10, Pool 10, Act 8 (Act also does consts + the u stores).
    seq = [0, 1, 2] * 8 + [0, 1, 0, 1]     # engine per load index

    with tile.TileContext(nc) as tc:
        with (
            tc.tile_pool(name="const", bufs=1) as const_pool,
            tc.tile_pool(name="zin", bufs=9) as zin_pool,
            tc.tile_pool(name="acc", bufs=2) as acc_pool,
            tc.tile_pool(name="psum", bufs=8, space="PSUM") as psum_pool,
        ):
            engs = [nc.sync, nc.gpsimd, nc.scalar]
            w1sb = const_pool.tile([P, 2 * H], f32)
            nc.scalar.dma_start(
                out=w1sb[:].rearrange("p (t h) -> p t h", h=H),
                in_=W1[:].rearrange("(t p) h -> p t h", p=P),
            )
            b1sb = const_pool.tile([P, H], f32)
            nc.scalar.dma_start(out=b1sb[:], in_=b1[:].to_broadcast([P, H]))

            jj = 0
            for t, zT in enumerate((z1T, z2T)):
                u_acc = acc_pool.tile([P, KN * H], f16, tag="u_acc")
                for j in range(NLOAD):
                    ztile = zin_pool.tile([P, CW], f32, tag="ztile")
                    engs[seq[jj]].dma_start(
                        out=ztile[:], in_=zT[:, j * CW:(j + 1) * CW])
                    jj += 1
                    ps = psum_pool.tile([P, CH * H], f32, tag="ps")
                    for i in range(CH):
                        nc.tensor.matmul(
                            out=ps[:, i * H:(i + 1) * H],
                            lhsT=ztile[:, i * P:(i + 1) * P],
                            rhs=w1sb[:, t * H:(t + 1) * H],
                            start=True, stop=True,
                        )
                    if t == 0:
                        nc.vector.tensor_tensor(
                            out=u_acc[:, j * CH * H:(j + 1) * CH * H],
                            in0=ps[:].rearrange("p (c h) -> p c h", h=H),
                            in1=b1sb[:].unsqueeze(1).to_broadcast([P, CH, H]),
                            op=mybir.AluOpType.add,
                        )
                    else:
                        nc.vector.tensor_copy(
                            out=u_acc[:, j * CH * H:(j + 1) * CH * H], in_=ps[:]
                        )
                nc.scalar.dma_start(out=u[t], in_=u_acc[:])
    nc.compile()
    return nc


# ---------------------------------------------------------------- kernel B


def build_scatter_kernel(cfg=CFG):
    """Per-core edge decoder via expansion + scatter-add + PE reduce.

    Inputs : U1c [NQ*ROWS1_Q, 8] f16   (pseudo-node table, quarter-major)
             U2c [NQ*ROWS2_Q, 8] f16   (per-quarter degree-bucketed table)
             idx [128, NQ*IDXW] i16    (scatter dst, 16-wrapped, replicated)
             W2 [1, 8] f32, b2 [1, 1] f32
    Output : out [128, NQ*KOUT] f16
             quarter q, slot t: hp = t // (SQ/2), t2 = t % (SQ/2),
             KH = KOUT/2 -> out[t2 // KH, q*KOUT + hp*KH + t2 % KH]
    """
    nc = _new_nc()
    R1 = cfg.NQ * cfg.ROWS1_Q
    R2 = cfg.NQ * cfg.ROWS2_Q
    U1c = nc.declare_dram_parameter("U1c", [R1, H], f16, isOutput=False)
    U2c = nc.declare_dram_parameter("U2c", [R2, H], f16, isOutput=False)
    idx = nc.declare_dram_parameter(
        "idx", [P, cfg.NQ * cfg.IDXW], i16, isOutput=False)
    W2 = nc.declare_dram_parameter("W2", [1, H], f16, isOutput=False)
    b2 = nc.declare_dram_parameter("b2", [1, 1], f16, isOutput=False)
    out = nc.declare_dram_parameter(
        "out", [P, cfg.NQ * cfg.KOUT], f16, isOutput=True)

    X = [
        nc.dram_tensor(f"Xscratch{q}", (cfg.SQ, cfg.XCOL), f16, kind="Internal")
        for q in range(cfg.NQ)
    ]

    with tile.TileContext(nc) as tc:
        with (
            tc.tile_pool(name="const", bufs=1) as const_pool,
            tc.tile_pool(name="tok", bufs=1) as tok_pool,
            tc.tile_pool(name="rb", bufs=3) as rb_pool,
        ):
            idxs = const_pool.tile([P, cfg.NQ * cfg.IDXW], i16)
            Y = [tok_pool.tile([P, cfg.KTOT * H], f16, tag=f"y{q}",
                               name=f"y{q}")
                 for q in range(cfg.NQ)]

            def load_idx(eng, q, c0=0, c1=None):
                c1 = cfg.IDXW if c1 is None else c1
                eng.dma_start(
                    out=idxs[:, q * cfg.IDXW + c0:q * cfg.IDXW + c1],
                    in_=idx[:, q * cfg.IDXW + c0:q * cfg.IDXW + c1])

            def expand_u1(eng, q):
                eng.dma_start(
                    out=X[q][:, 0:H].rearrange("(n d) h -> n d h", d=cfg.D1),
                    in_=U1c[q * cfg.ROWS1_Q:(q + 1) * cfg.ROWS1_Q, :]
                    .unsqueeze(1)
                    .to_broadcast([cfg.ROWS1_Q, cfg.D1, H]),
                )

            def expand_u2(eng, q, d):
                npp = cfg.CAP2[d] // 128
                kb = cfg.KB[d]
                kw = cfg.TOK2[d] // 128
                secbase = q * cfg.ROWS2_Q + (0 if d == 1 else cfg.CAP2[1])
                eng.dma_start(
                    out=Y[q][:, kb * H:(kb + kw) * H]
                    .rearrange("p (n d h) -> p n d h", d=d, h=H),
                    in_=U2c[secbase: secbase + cfg.CAP2[d], :]
                    .rearrange("(p n) h -> p n h", p=P)
                    .unsqueeze(2)
                    .to_broadcast([P, npp, d, H]),
                )

            # ---- phase 1, dependency-priority order ------------------
            # SP: X0 idx0 y1d1 idx2 y1d2 X3 y3d1; Act: idx1 X1 y2d1 idx3 X2
            # y2d2 y3d2 consts; Pool: y0d1 y0d2 then the scatter stream.
            ICH0 = cfg.CHUNK // 16
            load_idx(nc.sync, 0, 0, ICH0)
            load_idx(nc.scalar, 0, ICH0, None)
            expand_u1(nc.sync, 0)
            expand_u2(nc.gpsimd, 0, 1)
            expand_u2(nc.scalar, 0, 2)

            expand_u1(nc.scalar, 1)
            expand_u2(nc.sync, 1, 1)
            load_idx(nc.scalar, 1)

            load_idx(nc.sync, 2)
            expand_u2(nc.scalar, 2, 1)
            expand_u2(nc.sync, 1, 2)
            expand_u1(nc.sync, 2)

            load_idx(nc.scalar, 3)
            expand_u1(nc.sync, 3)
            expand_u2(nc.sync, 3, 1)
            expand_u2(nc.scalar, 2, 2)
            expand_u2(nc.sync, 3, 2)

            for q in range(cfg.NQ):
                if cfg.KUSED < cfg.KTOT:
                    nc.vector.memset(Y[q][:, cfg.KUSED * H:], 0.0)

            w2sb = const_pool.tile([P, H], f16)
            nc.scalar.dma_start(out=w2sb[:], in_=W2[:].to_broadcast([P, H]))
            b2sb = const_pool.tile([P, 1], f16)
            nc.scalar.dma_start(out=b2sb[:], in_=b2[:].to_broadcast([P, 1]))

            # ---- scatter-add tokens into X (Pool) --------------------
            KCH = cfg.CHUNK // 128
            ICH = cfg.CHUNK // 16
            for q in range(cfg.NQ):
                for j in range(cfg.NCHUNK):
                    nc.gpsimd.dma_scatter_add(
                        out_ap=X[q][:, 0:H],
                        in_ap=Y[q][:, j * KCH * H:(j + 1) * KCH * H]
                        .rearrange("p (k h) -> p k h", h=H),
                        idxs_ap=idxs[
                            :, q * cfg.IDXW + j * ICH:
                            q * cfg.IDXW + (j + 1) * ICH],
                        num_idxs=cfg.CHUNK,
                        num_idxs_reg=cfg.CHUNK,
                        elem_size=H,
                        elem_step=cfg.XCOL,
                        queue_num=0,
                    )

            # ---- readback + fused relu*|W2| + signed tree-reduce (+b2)
            KH = cfg.KOUT // 2                   # out cols per half piece
            rb_engs = [nc.gpsimd, nc.sync, nc.scalar, nc.sync,
                       nc.scalar, nc.sync, nc.scalar, nc.sync]
            out_acc = const_pool.tile([P, cfg.NQ * cfg.KOUT], f16)
            for q in range(cfg.NQ):
                for hp in range(2):
                    piece = q * 2 + hp
                    rb = rb_pool.tile([P, KH * H], f16, tag="rb")
                    r0 = hp * (cfg.SQ // 2)
                    rb_engs[piece].dma_start(
                        out=rb[:].rearrange("p (k h) -> p k h", h=H),
                        in_=X[q][r0:r0 + cfg.SQ // 2, 0:H]
                        .rearrange("(p k) h -> p k h", p=P),
                    )
                    # relu on DVE (4x f16), then *W2 broadcast (2x)
                    nc.vector.tensor_scalar(
                        out=rb[:], in0=rb[:], scalar1=0.0, scalar2=None,
                        op0=mybir.AluOpType.max)
                    meng = nc.vector
                    prod = rb_pool.tile([P, KH * H], f16, tag="prod")
                    meng.tensor_tensor(
                        out=prod[:].rearrange("p (k h) -> p k h", h=H),
                        in0=rb[:].rearrange("p (k h) -> p k h", h=H),
                        in1=w2sb[:].unsqueeze(1).to_broadcast([P, KH, H]),
                        op=mybir.AluOpType.mult,
                    )
                    pv = prod[:].rearrange("p (k h) -> p k h", h=H)
                    t1 = rb_pool.tile([P, KH * 4], f16, tag="t1")
                    meng.tensor_tensor(
                        out=t1[:].rearrange("p (k h) -> p k h", h=4),
                        in0=pv[:, :, 0:4], in1=pv[:, :, 4:8],
                        op=mybir.AluOpType.add,
                    )
                    t1v = t1[:].rearrange("p (k h) -> p k h", h=4)
                    t2 = rb_pool.tile([P, KH * 2], f16, tag="t2")
                    teng = nc.gpsimd
                    teng.tensor_tensor(
                        out=t2[:].rearrange("p (k h) -> p k h", h=2),
                        in0=t1v[:, :, 0:2], in1=t1v[:, :, 2:4],
                        op=mybir.AluOpType.add,
                    )
                    t2v = t2[:].rearrange("p (k h) -> p k h", h=2)
                    t3 = rb_pool.tile([P, KH], f16, tag="t3")
                    teng.tensor_tensor(
                        out=t3[:].rearrange("p (k h) -> p k h", h=1),
                        in0=t2v[:, :, 0:1], in1=t2v[:, :, 1:2],
                        op=mybir.AluOpType.add,
                    )
                    teng.tensor_tensor(
                        out=out_acc[:, q * cfg.KOUT + hp * KH:
                                    q * cfg.KOUT + (hp + 1) * KH],
                        in0=t3[:], in1=b2sb[:].to_broadcast([P, KH]),
                        op=mybir.AluOpType.add,
                    )
                if q == 1:
                    nc.scalar.dma_start(
                        out=out[:, 0:2 * cfg.KOUT],
                        in_=out_acc[:, 0:2 * cfg.KOUT])
            nc.sync.dma_start(
                out=out[:, 2 * cfg.KOUT:], in_=out_acc[:, 2 * cfg.KOUT:])
    nc.compile()
    return nc


# ------------------------------------------------------------ host planning


def plan_core(rows, cols, cfg=CFG):
    """Plan one core's slot/token layout.

    rows, cols: int arrays [E_c] of node ids (r in core's range, c global)
    for the core's edges, in original edge order.
    """
    E = len(rows)
    assert len(cols) == E

    # ---- u1 side: group edges by row node, split into deg<=D1 pseudo-nodes
    order = np.argsort(rows, kind="stable")
    srows = rows[order]
    uniq, starts = np.unique(srows, return_index=True)
    ends = np.append(starts[1:], E)

    pseudo_node = []
    pseudo_edges = []
    for n, s, e in zip(uniq, starts, ends):
        ecnt = e - s
        for off in range(0, ecnt, cfg.D1):
            pseudo_node.append(n)
            pseudo_edges.append(order[s + off: s + off + min(cfg.D1, ecnt - off)])
    NPN = len(pseudo_node)
    assert NPN <= cfg.NQ * cfg.PN_Q, (NPN, cfg.NQ * cfg.PN_Q)
    pnq = -(-NPN // cfg.NQ)                       # balanced quarter split
    assert pnq <= cfg.PN_Q

    u1_rows = np.full(cfg.NQ * cfg.ROWS1_Q, -1, dtype=np.int64)
    slot_of = np.full(E, -1, dtype=np.int64)
    for q in range(cfg.NQ):
        lo, hi = q * pnq, min((q + 1) * pnq, NPN)
        for slot_j, i in enumerate(range(lo, hi)):
            u1_rows[q * cfg.ROWS1_Q + slot_j] = pseudo_node[i]
            base = q * cfg.SQ + slot_j * cfg.D1
            for t, eid in enumerate(pseudo_edges[i]):
                slot_of[eid] = base + t
    assert (slot_of >= 0).all()

    # ---- u2 side: per quarter, bucket cols by in-quarter degree (chunks <=2)
    quarter_of = slot_of // cfg.SQ
    u2_rows = np.full(cfg.NQ * cfg.ROWS2_Q, -1, dtype=np.int64)
    idx16 = np.full((16, cfg.NQ * cfg.IDXW), 0, dtype=np.int16)
    trash_local = cfg.SQ - 1
    for q in range(cfg.NQ):
        idx16[:, q * cfg.IDXW:(q + 1) * cfg.IDXW] = trash_local

    for q in range(cfg.NQ):
        m = quarter_of == q
        qcols = cols[m]
        qeids = np.nonzero(m)[0]
        order_c = np.argsort(qcols, kind="stable")
        sc = qcols[order_c]
        uniq_c, st_c = np.unique(sc, return_index=True)
        en_c = np.append(st_c[1:], len(sc))
        chunks = {d: [] for d in cfg.BUCKETS}
        mx = cfg.BUCKETS[-1]
        for n, s, e in zip(uniq_c, st_c, en_c):
            for off in range(s, e, mx):
                grp = order_c[off: min(off + mx, e)]
                chunks[len(grp)].append((n, qeids[grp]))
        rowbase = q * cfg.ROWS2_Q
        for d in cfg.BUCKETS:
            cap = cfg.CAP2[d]
            lst = chunks[d]
            assert len(lst) <= cap, (q, d, len(lst), cap)
            npp = cap // 128
            for slot_j, (n, eids) in enumerate(lst):
                u2_rows[rowbase + slot_j] = n
                pp = slot_j // npp
                kk = cfg.KB[d] + (slot_j % npp) * d
                for rep, eid in enumerate(eids):
                    i_tok = (kk + rep) * 128 + pp
                    local = slot_of[eid] - q * cfg.SQ
                    idx16[i_tok % 16, q * cfg.IDXW + i_tok // 16] = local
            rowbase += cap

    return {
        "u1_rows": u1_rows,
        "u2_rows": u2_rows,
        "idx": idx16,
        "slot_of": slot_of,
    }


def shard_edges(edge_row):
    """r-quantile sharding: (node_hi[8], edge_core[E])."""
    counts = np.bincount(edge_row, minlength=N_NODES)
    cum = np.cumsum(counts)
    node_hi = np.zeros(N_CORES, dtype=np.int64)
    tgt = N_EDGES / N_CORES
    for c in range(N_CORES - 1):
        node_hi[c] = np.searchsorted(cum, tgt * (c + 1))
    node_hi[N_CORES - 1] = N_NODES
    edge_core = np.searchsorted(node_hi, edge_row, side="right")
    return node_hi, edge_core


# --------------------------------------------------------------- host glue


def gather_u(uA, table, nodes):
    """uA: per-core [2, 128, KN*8] f16 arrays. Returns [len(nodes), 8] f16,
    zeros where nodes == -1."""
    res = np.zeros((len(nodes), H), dtype=np.float16)
    valid = nodes >= 0
    n = nodes[valid]
    c = n // NC_NODES
    m = n % NC_NODES
    p, k = m % P, m // P
    vals = np.empty((len(n), H), dtype=np.float16)
    for cc in range(N_CORES):
        sel = c == cc
        if sel.any():
            tbl = uA[cc][table].reshape(P, KN, H)
            vals[sel] = tbl[p[sel], k[sel]]
    res[valid] = vals
    return res


def prep_precompute_inputs(z1, z2, W1, b1):
    W1 = np.ascontiguousarray(W1, dtype=np.float32)
    b1 = np.ascontiguousarray(b1, dtype=np.float32).reshape(1, H)
    in_maps = []
    for c in range(N_CORES):
        m = {}
        for name, z in (("z1T", z1), ("z2T", z2)):
            sh = np.zeros((NP, D), dtype=np.float32)
            sh[:NC_NODES] = z[c * NC_NODES:(c + 1) * NC_NODES]
            m[name] = np.ascontiguousarray(sh.T)
        m["W1"] = W1
        m["b1"] = b1
        in_maps.append(m)
    return in_maps


_CACHE = {}


def _get_kernels():
    if "a" not in _CACHE:
        _CACHE["a"] = build_precompute()
        _CACHE["b"] = build_scatter_kernel()
    return _CACHE["a"], _CACHE["b"]


LAST_IN_MAPS = {}


def kernel(z1, z2, edge_index, W1, b1, W2, b2):
    z1 = np.asarray(z1, dtype=np.float32)
    z2 = np.asarray(z2, dtype=np.float32)
    edge_index = np.asarray(edge_index)
    W2 = np.ascontiguousarray(np.asarray(W2, dtype=np.float32)
                              .reshape(1, H).astype(np.float16))
    b2 = np.ascontiguousarray(np.asarray(b2, dtype=np.float32)
                              .reshape(1, 1).astype(np.float16))
    cfg = CFG

    nc_a, nc_b = _get_kernels()
    core_ids = list(range(N_CORES))

    # ---- launch A
    in_maps_a = prep_precompute_inputs(z1, z2, W1, b1)
    LAST_IN_MAPS["a"] = in_maps_a
    res_a = run_bass_kernel_spmd(nc_a, in_maps_a, core_ids)
    uA = [res_a.results[c]["u"] for c in range(N_CORES)]

    # ---- plan + launch B
    row = np.asarray(edge_index[0], dtype=np.int64)
    col = np.asarray(edge_index[1], dtype=np.int64)
    node_hi, edge_core = shard_edges(row)

    in_maps, plans, eids = [], [], []
    for c in range(N_CORES):
        m = edge_core == c
        eid = np.nonzero(m)[0]
        plan = plan_core(row[eid], col[eid], cfg)
        U1c = gather_u(uA, 0, plan["u1_rows"])
        U2c = gather_u(uA, 1, plan["u2_rows"])
        in_maps.append({
            "U1c": np.ascontiguousarray(U1c),
            "U2c": np.ascontiguousarray(U2c),
            "idx": np.ascontiguousarray(np.tile(plan["idx"], (8, 1))),
            "W2": W2,
            "b2": b2,
        })
        plans.append(plan)
        eids.append(eid)
    LAST_IN_MAPS["b"] = in_maps
    res_b = run_bass_kernel_spmd(nc_b, in_maps, core_ids)

    # ---- unpermute (device out is f16; upcast on host)
    out = np.empty((N_EDGES, 1), dtype=np.float32)
    for c in range(N_CORES):
        o = res_b.results[c]["out"]          # [128, NQ*KOUT] f16
        vals = slot_values(o, plans[c]["slot_of"], cfg)
        out[eids[c], 0] = vals.astype(np.float32)
    return out


def slot_values(o, slot, cfg=CFG):
    """Map kernel B 'out' [128, NQ*KOUT] to per-slot values."""
    KH = cfg.KOUT // 2
    q, t = slot // cfg.SQ, slot % cfg.SQ
    hp, t2 = t // (cfg.SQ // 2), t % (cfg.SQ // 2)
    return o[t2 // KH, q * cfg.KOUT + hp * KH + t2 % KH]


# revision 38
# speedup vs baseline: 1.0028x; 1.0028x over previous
"""Trainium2 Bass kernel for nn_DirectedEdgeDecoder (gnn_message_passing) — v3.1.

  out[e] = W2 . relu(concat(z1[row_e], z2[col_e]) @ W1 + b1) + b2

Decomposition: per-node projections u1 = z1 @ W1[:D] + b1, u2 = z2 @ W1[D:]
(8 f16 values/node, kernel A), then per-edge combine (kernel B):
r-quantile edge sharding; u1 expanded into a DRAM slot array X by
pseudo-node (deg<=2) broadcast DMAs; u2 expanded into SBUF token streams by
in-quarter col-degree buckets {1,2}; bulk dma_scatter_add routes each token
onto its edge slot; strided readback + relu + .W2 + tree-reduce.

Scheduling (cost-model driven): DMA work balanced across the three
DMA-capable engines (SP / Act / Pool-SWDGE), ordered so quarter q's scatter
dependencies (idx, X-expansion, token stream) complete just in time; the
whole per-slot compute chain runs in f16 with DVE 2x/4x perf modes, with
tail ops placed on Pool/Act to keep DVE the only near-critical engine.
"""
import numpy as np
import concourse.bass as bass
import concourse.mybir as mybir
import concourse.tile as tile
from concourse import bacc
from concourse.bass_utils import run_bass_kernel_spmd

P = 128
N_CORES = 8
N_NODES = 100000
N_EDGES = 800000
D = 128
H = 8

f32 = mybir.dt.float32
f16 = mybir.dt.float16
i16 = mybir.dt.int16

# ------------------------------------------------------------- configuration


class Cfg:
    """Geometry of kernel B. All sizes are compile-time constants."""

    def __init__(self):
        self.NQ = 4                       # slot quarters per core
        self.D1 = 2                       # u1 pseudo-node degree
        self.PN_Q = 13376                 # u1 pseudo-node cap per quarter
        self.TRASH = 128                  # trash slots per quarter
        self.CAP2 = {1: 20096, 2: 2944}   # u2 chunk caps per quarter
        self.NCHUNK = 4                   # scatter instructions per quarter
        self.SQ = self.PN_Q * self.D1 + self.TRASH    # slots / quarter
        assert self.SQ % 256 == 0 and self.SQ < 32768
        self.S = self.NQ * self.SQ
        self.BUCKETS = [1, 2]
        self.TOK2 = {d: c * d for d, c in self.CAP2.items()}
        assert all(v % 128 == 0 for v in self.TOK2.values())
        self.NTOK_USED = sum(self.TOK2.values())      # 25984
        self.KUSED = self.NTOK_USED // 128            # 203
        self.KTOT = 204
        self.NTOKQ = self.KTOT * 128                  # 26624
        self.CHUNK = self.NTOKQ // self.NCHUNK        # 6656 tokens
        assert self.CHUNK % 128 == 0, self.CHUNK
        self.KB = {}
        kb = 0
        for d in self.BUCKETS:
            self.KB[d] = kb
            kb += self.TOK2[d] // 128
        assert kb == self.KUSED
        self.ROWS2_Q = sum(self.CAP2.values())        # U2c rows per quarter
        assert self.TRASH % self.D1 == 0
        self.ROWS1_Q = self.PN_Q + self.TRASH // self.D1
        self.XCOL = 128                   # f16 per X row (256B stride)
        self.IDXW = self.NTOKQ // 16      # idx cols per quarter (1664)
        self.KOUT = self.SQ // 128        # output cols per quarter (212)


CFG = Cfg()

# ---------------------------------------------------------------- kernel A

NC_NODES = N_NODES // N_CORES          # 12500 nodes per core
KN = 98                                # node blocks per core
NP = KN * P                            # 12544 padded nodes per core


def _new_nc():
    return bacc.Bacc(
        "TRN2", target_bir_lowering=False, debug=False, num_devices=N_CORES,
        num_swdge_queues=1,
    )


def build_precompute():
    """Per-core: u[t] = zT[t].T @ W1[t*128:(t+1)*128] (+ b1 if t == 0), f16.

    Inputs : z1T [128, NP] f32 (shard, transposed, padded), z2T likewise,
             W1 [256, 8] f32, b1 [1, 8] f32
    Output : u [2, 128, KN*8] f16 -- u[t][p, k*8+h] = node (k*128+p) proj h
    """
    nc = _new_nc()
    z1T = nc.declare_dram_parameter("z1T", [P, NP], f32, isOutput=False)
    z2T = nc.declare_dram_parameter("z2T", [P, NP], f32, isOutput=False)
    W1 = nc.declare_dram_parameter("W1", [2 * D, H], f32, isOutput=False)
    b1 = nc.declare_dram_parameter("b1", [1, H], f32, isOutput=False)
    u = nc.declare_dram_parameter("u", [2, P, KN * H], f16, isOutput=True)

    CH = 7
    CW = CH * P
    NLOAD = KN // CH                       # 14 loads per table

    # consts go first on Act; z loads start immediately on SP/Pool.
    # 28 loads: SP 10, Pool 10, Act 8 (Act also does consts + the u stores).
    seq = [0, 1, 2] * 8 + [0, 1, 0, 1]     # engine per load index

    with tile.TileContext(nc) as tc:
        with (
            tc.tile_pool(name="const", bufs=1) as const_pool,
            tc.tile_pool(name="zin", bufs=9) as zin_pool,
            tc.tile_pool(name="acc", bufs=2) as acc_pool,
            tc.tile_pool(name="psum", bufs=8, space="PSUM") as psum_pool,
        ):
            engs = [nc.sync, nc.gpsimd, nc.scalar]
            w1sb = const_pool.tile([P, 2 * H], f32)
            nc.scalar.dma_start(
                out=w1sb[:].rearrange("p (t h) -> p t h", h=H),
                in_=W1[:].rearrange("(t p) h -> p t h", p=P),
            )
            b1sb = const_pool.tile([P, H], f32)
            nc.scalar.dma_start(out=b1sb[:], in_=b1[:].to_broadcast([P, H]))

            jj = 0
            for t, zT in enumerate((z1T, z2T)):
                u_acc = acc_pool.tile([P, KN * H], f16, tag="u_acc")
                for j in range(NLOAD):
                    ztile = zin_pool.tile([P, CW], f32, tag="ztile")
                    engs[seq[jj]].dma_start(
                        out=ztile[:], in_=zT[:, j * CW:(j + 1) * CW])
                    jj += 1
                    ps = psum_pool.tile([P, CH * H], f32, tag="ps")
                    for i in range(CH):
                        nc.tensor.matmul(
                            out=ps[:, i * H:(i + 1) * H],
                            lhsT=ztile[:, i * P:(i + 1) * P],
                            rhs=w1sb[:, t * H:(t + 1) * H],
                            start=True, stop=True,
                        )
                    if t == 0:
                        nc.vector.tensor_tensor(
                            out=u_acc[:, j * CH * H:(j + 1) * CH * H],
                            in0=ps[:].rearrange("p (c h) -> p c h", h=H),
                            in1=b1sb[:].unsqueeze(1).to_broadcast([P, CH, H]),
                            op=mybir.AluOpType.add,
                        )
                    else:
                        nc.vector.tensor_copy(
                            out=u_acc[:, j * CH * H:(j + 1) * CH * H], in_=ps[:]
                        )
                nc.scalar.dma_start(out=u[t], in_=u_acc[:])
    nc.compile()
    return nc


# ---------------------------------------------------------------- kernel B


def build_scatter_kernel(cfg=CFG):
    """Per-core edge decoder via expansion + scatter-add + PE reduce.

    Inputs : U1c [NQ*ROWS1_Q, 8] f16   (pseudo-node table, quarter-major)
             U2c [NQ*ROWS2_Q, 8] f16   (per-quarter degree-bucketed table)
             idx [128, NQ*IDXW] i16    (scatter dst, 16-wrapped, replicated)
             W2 [1, 8] f32, b2 [1, 1] f32
    Output : out [128, NQ*KOUT] f16
             quarter q, slot t: hp = t // (SQ/2), t2 = t % (SQ/2),
             KH = KOUT/2 -> out[t2 // KH, q*KOUT + hp*KH + t2 % KH]
    """
    nc = _new_nc()
    R1 = cfg.NQ * cfg.ROWS1_Q
    R2 = cfg.NQ * cfg.ROWS2_Q
    U1c = nc.declare_dram_parameter("U1c", [R1, H], f16, isOutput=False)
    U2c = nc.declare_dram_parameter("U2c", [R2, H], f16, isOutput=False)
    idx = nc.declare_dram_parameter(
        "idx", [P, cfg.NQ * cfg.IDXW], i16, isOutput=False)
    W2 = nc.declare_dram_parameter("W2", [1, H], f16, isOutput=False)
    b2 = nc.declare_dram_parameter("b2", [1, 1], f16, isOutput=False)
    out = nc.declare_dram_parameter(
        "out", [P, cfg.NQ * cfg.KOUT], f16, isOutput=True)

    X = [
        nc.dram_tensor(f"Xscratch{q}", (cfg.SQ, cfg.XCOL), f16, kind="Internal")
        for q in range(cfg.NQ)
    ]

    with tile.TileContext(nc) as tc:
        with (
            tc.tile_pool(name="const", bufs=1) as const_pool,
            tc.tile_pool(name="tok", bufs=1) as tok_pool,
            tc.tile_pool(name="rb", bufs=3) as rb_pool,
        ):
            idxs = const_pool.tile([P, cfg.NQ * cfg.IDXW], i16)
            Y = [tok_pool.tile([P, cfg.KTOT * H], f16, tag=f"y{q}",
                               name=f"y{q}")
                 for q in range(cfg.NQ)]

            def load_idx(eng, q, c0=0, c1=None):
                c1 = cfg.IDXW if c1 is None else c1
                eng.dma_start(
                    out=idxs[:, q * cfg.IDXW + c0:q * cfg.IDXW + c1],
                    in_=idx[:, q * cfg.IDXW + c0:q * cfg.IDXW + c1])

            def expand_u1(eng, q):
                eng.dma_start(
                    out=X[q][:, 0:H].rearrange("(n d) h -> n d h", d=cfg.D1),
                    in_=U1c[q * cfg.ROWS1_Q:(q + 1) * cfg.ROWS1_Q, :]
                    .unsqueeze(1)
                    .to_broadcast([cfg.ROWS1_Q, cfg.D1, H]),
                )

            def expand_u2(eng, q, d):
                npp = cfg.CAP2[d] // 128
                kb = cfg.KB[d]
                kw = cfg.TOK2[d] // 128
                secbase = q * cfg.ROWS2_Q + (0 if d == 1 else cfg.CAP2[1])
                eng.dma_start(
                    out=Y[q][:, kb * H:(kb + kw) * H]
                    .rearrange("p (n d h) -> p n d h", d=d, h=H),
                    in_=U2c[secbase: secbase + cfg.CAP2[d], :]
                    .rearrange("(p n) h -> p n h", p=P)
                    .unsqueeze(2)
                    .to_broadcast([P, npp, d, H]),
                )

            # ---- phase 1, dependency-priority order ------------------
            # SP: X0 idx0 y1d1 idx2 y1d2 X3 y3d1; Act: idx1 X1 y2d1 idx3 X2
            # y2d2 y3d2 consts; Pool: y0d1 y0d2 then the scatter stream.
            ICH0 = cfg.CHUNK // 16
            load_idx(nc.sync, 0, 0, ICH0)
            load_idx(nc.scalar, 0, ICH0, None)
            expand_u1(nc.sync, 0)
            expand_u2(nc.gpsimd, 0, 1)
            expand_u2(nc.scalar, 0, 2)

            expand_u1(nc.scalar, 1)
            expand_u2(nc.sync, 1, 1)
            load_idx(nc.scalar, 1)

            load_idx(nc.sync, 2)
            expand_u2(nc.scalar, 2, 1)
            expand_u2(nc.sync, 1, 2)
            expand_u1(nc.sync, 2)

            load_idx(nc.scalar, 3)
            expand_u1(nc.sync, 3)
            expand_u2(nc.sync, 3, 1)
            expand_u2(nc.scalar, 2, 2)
            expand_u2(nc.sync, 3, 2)

            for q in range(cfg.NQ):
                if cfg.KUSED < cfg.KTOT:
                    nc.vector.memset(Y[q][:, cfg.KUSED * H:], 0.0)

            w2sb = const_pool.tile([P, H], f16)
            nc.scalar.dma_start(out=w2sb[:], in_=W2[:].to_broadcast([P, H]))
            b2sb = const_pool.tile([P, 1], f16)
            nc.scalar.dma_start(out=b2sb[:], in_=b2[:].to_broadcast([P, 1]))

            # ---- scatter-add tokens into X (Pool) --------------------
            KCH = cfg.CHUNK // 128
            ICH = cfg.CHUNK // 16
            for q in range(cfg.NQ):
                for j in range(cfg.NCHUNK):
                    nc.gpsimd.dma_scatter_add(
                        out_ap=X[q][:, 0:H],
                        in_ap=Y[q][:, j * KCH * H:(j + 1) * KCH * H]
                        .rearrange("p (k h) -> p k h", h=H),
                        idxs_ap=idxs[
                            :, q * cfg.IDXW + j * ICH:
                            q * cfg.IDXW + (j + 1) * ICH],
                        num_idxs=cfg.CHUNK,
                        num_idxs_reg=cfg.CHUNK,
                        elem_size=H,
                        elem_step=cfg.XCOL,
                        queue_num=0,
                    )

            # ---- readback + fused relu*|W2| + signed tree-reduce (+b2)
            KH = cfg.KOUT // 2                   # out cols per half piece
            rb_engs = [nc.gpsimd, nc.sync, nc.scalar, nc.sync,
                       nc.scalar, nc.sync, nc.scalar, nc.sync]
            out_acc = const_pool.tile([P, cfg.NQ * cfg.KOUT], f16)
            for q in range(cfg.NQ):
                for hp in range(2):
                    piece = q * 2 + hp
                    rb = rb_pool.tile([P, KH * H], f16, tag="rb")
                    r0 = hp * (cfg.SQ // 2)
                    rb_engs[piece].dma_start(
                        out=rb[:].rearrange("p (k h) -> p k h", h=H),
                        in_=X[q][r0:r0 + cfg.SQ // 2, 0:H]
                        .rearrange("(p k) h -> p k h", p=P),
                    )
                    # relu on DVE (4x f16), then *W2 broadcast (2x)
                    nc.vector.tensor_scalar(
                        out=rb[:], in0=rb[:], scalar1=0.0, scalar2=None,
                        op0=mybir.AluOpType.max)
                    meng = nc.vector
                    prod = rb_pool.tile([P, KH * H], f16, tag="prod")
                    meng.tensor_tensor(
                        out=prod[:].rearrange("p (k h) -> p k h", h=H),
                        in0=rb[:].rearrange("p (k h) -> p k h", h=H),
                        in1=w2sb[:].unsqueeze(1).to_broadcast([P, KH, H]),
                        op=mybir.AluOpType.mult,
                    )
                    pv = prod[:].rearrange("p (k h) -> p k h", h=H)
                    t1 = rb_pool.tile([P, KH * 4], f16, tag="t1")
                    meng.tensor_tensor(
                        out=t1[:].rearrange("p (k h) -> p k h", h=4),
                        in0=pv[:, :, 0:4], in1=pv[:, :, 4:8],
                        op=mybir.AluOpType.add,
                    )
                    t1v = t1[:].rearrange("p (k h) -> p k h", h=4)
                    t2 = rb_pool.tile([P, KH * 2], f16, tag="t2")
                    teng = nc.gpsimd
                    teng.tensor_tensor(
                        out=t2[:].rearrange("p (k h) -> p k h", h=2),
                        in0=t1v[:, :, 0:2], in1=t1v[:, :, 2:4],
                        op=mybir.AluOpType.add,
                    )
                    t2v = t2[:].rearrange("p (k h) -> p k h", h=2)
                    t3 = rb_pool.tile([P, KH], f16, tag="t3")
                    teng.tensor_tensor(
                        out=t3[:].rearrange("p (k h) -> p k h", h=1),
                        in0=t2v[:, :, 0:1], in1=t2v[:, :, 1:2],
                        op=mybir.AluOpType.add,
                    )
                    teng.tensor_tensor(
                        out=out_acc[:, q * cfg.KOUT + hp * KH:
                                    q * cfg.KOUT + (hp + 1) * KH],
                        in0=t3[:], in1=b2sb[:].to_broadcast([P, KH]),
                        op=mybir.AluOpType.add,
                    )
                if q == 1:
                    nc.scalar.dma_start(
                        out=out[:, 0:2 * cfg.KOUT],
                        in_=out_acc[:, 0:2 * cfg.KOUT])
            nc.sync.dma_start(
                out=out[:, 2 * cfg.KOUT:], in_=out_acc[:, 2 * cfg.KOUT:])
    nc.compile()
    return nc


# ------------------------------------------------------------ host planning


def plan_core(rows, cols, cfg=CFG):
    """Plan one core's slot/token layout.

    rows, cols: int arrays [E_c] of node ids (r in core's range, c global)
    for the core's edges, in original edge order.
    """
    E = len(rows)
    assert len(cols) == E

    # ---- u1 side: group edges by row node, split into deg<=D1 pseudo-nodes
    order = np.argsort(rows, kind="stable")
    srows = rows[order]
    uniq, starts = np.unique(srows, return_index=True)
    ends = np.append(starts[1:], E)

    pseudo_node = []
    pseudo_edges = []
    for n, s, e in zip(uniq, starts, ends):
        ecnt = e - s
        for off in range(0, ecnt, cfg.D1):
            pseudo_node.append(n)
            pseudo_edges.append(order[s + off: s + off + min(cfg.D1, ecnt - off)])
    NPN = len(pseudo_node)
    assert NPN <= cfg.NQ * cfg.PN_Q, (NPN, cfg.NQ * cfg.PN_Q)
    pnq = -(-NPN // cfg.NQ)                       # balanced quarter split
    assert pnq <= cfg.PN_Q

    u1_rows = np.full(cfg.NQ * cfg.ROWS1_Q, -1, dtype=np.int64)
    slot_of = np.full(E, -1, dtype=np.int64)
    for q in range(cfg.NQ):
        lo, hi = q * pnq, min((q + 1) * pnq, NPN)
        for slot_j, i in enumerate(range(lo, hi)):
            u1_rows[q * cfg.ROWS1_Q + slot_j] = pseudo_node[i]
            base = q * cfg.SQ + slot_j * cfg.D1
            for t, eid in enumerate(pseudo_edges[i]):
                slot_of[eid] = base + t
    assert (slot_of >= 0).all()

    # ---- u2 side: per quarter, bucket cols by in-quarter degree (chunks <=2)
    quarter_of = slot_of // cfg.SQ
    u2_rows = np.full(cfg.NQ * cfg.ROWS2_Q, -1, dtype=np.int64)
    idx16 = np.full((16, cfg.NQ * cfg.IDXW), 0, dtype=np.int16)
    trash_local = cfg.SQ - 1
    for q in range(cfg.NQ):
        idx16[:, q * cfg.IDXW:(q + 1) * cfg.IDXW] = trash_local

    for q in range(cfg.NQ):
        m = quarter_of == q
        qcols = cols[m]
        qeids = np.nonzero(m)[0]
        order_c = np.argsort(qcols, kind="stable")
        sc = qcols[order_c]
        uniq_c, st_c = np.unique(sc, return_index=True)
        en_c = np.append(st_c[1:], len(sc))
        chunks = {d: [] for d in cfg.BUCKETS}
        mx = cfg.BUCKETS[-1]
        for n, s, e in zip(uniq_c, st_c, en_c):
            for off in range(s, e, mx):
                grp = order_c[off: min(off + mx, e)]
                chunks[len(grp)].append((n, qeids[grp]))
        rowbase = q * cfg.ROWS2_Q
        for d in cfg.BUCKETS:
            cap = cfg.CAP2[d]
            lst = chunks[d]
            assert len(lst) <= cap, (q, d, len(lst), cap)
            npp = cap // 128
            for slot_j, (n, eids) in enumerate(lst):
                u2_rows[rowbase + slot_j] = n
                pp = slot_j // npp
                kk = cfg.KB[d] + (slot_j % npp) * d
                for rep, eid in enumerate(eids):
                    i_tok = (kk + rep) * 128 + pp
                    local = slot_of[eid] - q * cfg.SQ
                    idx16[i_tok % 16, q * cfg.IDXW + i_tok // 16] = local
            rowbase += cap

    return {
        "u1_rows": u1_rows,
        "u2_rows": u2_rows,
        "idx": idx16,
        "slot_of": slot_of,
    }


def shard_edges(edge_row):
    """r-quantile sharding: (node_hi[8], edge_core[E])."""
    counts = np.bincount(edge_row, minlength=N_NODES)
    cum = np.cumsum(counts)
    node_hi = np.zeros(N_CORES, dtype=np.int64)
    tgt = N_EDGES / N_CORES
    for c in range(N_CORES - 1):
        node_hi[c] = np.searchsorted(cum, tgt * (c + 1))
    node_hi[N_CORES - 1] = N_NODES
    edge_core = np.searchsorted(node_hi, edge_row, side="right")
    return node_hi, edge_core


# --------------------------------------------------------------- host glue


def gather_u(uA, table, nodes):
    """uA: per-core [2, 128, KN*8] f16 arrays. Returns [len(nodes), 8] f16,
    zeros where nodes == -1."""
    res = np.zeros((len(nodes), H), dtype=np.float16)
    valid = nodes >= 0
    n = nodes[valid]
    c = n // NC_NODES
    m = n % NC_NODES
    p, k = m % P, m // P
    vals = np.empty((len(n), H), dtype=np.float16)
    for cc in range(N_CORES):
        sel = c == cc
        if sel.any():
            tbl = uA[cc][table].reshape(P, KN, H)
            vals[sel] = tbl[p[sel], k[sel]]
    res[valid] = vals
    return res


def prep_precompute_inputs(z1, z2, W1, b1):
    W1 = np.ascontiguousarray(W1, dtype=np.float32)
    b1 = np.ascontiguousarray(b1, dtype=np.float32).reshape(1, H)
    in_maps = []
    for c in range(N_CORES):
        m = {}
        for name, z in (("z1T", z1), ("z2T", z2)):
            sh = np.zeros((NP, D), dtype=np.float32)
            sh[:NC_NODES] = z[c * NC_NODES:(c + 1) * NC_NODES]
            m[name] = np.ascontiguousarray(sh.T)
        m["W1"] = W1
        m["b1"] = b1
        in_maps.append(m)
    return in_maps


_CACHE = {}


def _get_kernels():
    if "a" not in _CACHE:
        _CACHE["a"] = build_precompute()
        _CACHE["b"] = build_scatter_kernel()
    return _CACHE["a"], _CACHE["b"]


LAST_IN_MAPS = {}


def kernel(z1, z2, edge_index, W1, b1, W2, b2):
    z1 = np.asarray(z1, dtype=np.float32)
    z2 = np.asarray(z2, dtype=np.float32)
    edge_index = np.asarray(edge_index)
    W2 = np.ascontiguousarray(np.asarray(W2, dtype=np.float32)
                              .reshape(1, H).astype(np.float16))
    b2 = np.ascontiguousarray(np.asarray(b2, dtype=np.float32)
                              .reshape(1, 1).astype(np.float16))
    cfg = CFG

    nc_a, nc_b = _get_kernels()
    core_ids = list(range(N_CORES))

    # ---- launch A
    in_maps_a = prep_precompute_inputs(z1, z2, W1, b1)
    LAST_IN_MAPS["a"] = in_maps_a
    res_a = run_bass_kernel_spmd(nc_a, in_maps_a, core_ids)
    uA = [res_a.results[c]["u"] for c in range(N_CORES)]

    # ---- plan + launch B
    row = np.asarray(edge_index[0], dtype=np.int64)
    col = np.asarray(edge_index[1], dtype=np.int64)
    node_hi, edge_core = shard_edges(row)

    in_maps, plans, eids = [], [], []
    for c in range(N_CORES):
        m = edge_core == c
        eid = np.nonzero(m)[0]
        plan = plan_core(row[eid], col[eid], cfg)
        U1c = gather_u(uA, 0, plan["u1_rows"])
        U2c = gather_u(uA, 1, plan["u2_rows"])
        in_maps.append({
            "U1c": np.ascontiguousarray(U1c),
            "U2c": np.ascontiguousarray(U2c),
            "idx": np.ascontiguousarray(np.tile(plan["idx"], (8, 1))),
            "W2": W2,
            "b2": b2,
        })
        plans.append(plan)
        eids.append(eid)
    LAST_IN_MAPS["b"] = in_maps
    res_b = run_bass_kernel_spmd(nc_b, in_maps, core_ids)

    # ---- unpermute (device out is f16; upcast on host)
    out = np.empty((N_EDGES, 1), dtype=np.float32)
    for c in range(N_CORES):
        o = res_b.results[c]["out"]          # [128, NQ*KOUT] f16
        vals = slot_values(o, plans[c]["slot_of"], cfg)
        out[eids[c], 0] = vals.astype(np.float32)
    return out


def slot_values(o, slot, cfg=CFG):
    """Map kernel B 'out' [128, NQ*KOUT] to per-slot values."""
    KH = cfg.KOUT // 2
    q, t = slot // cfg.SQ, slot % cfg.SQ
    hp, t2 = t // (cfg.SQ // 2), t % (cfg.SQ // 2)
    return o[t2 // KH, q * cfg.KOUT + hp * KH + t2 % KH]


# revision 39
# speedup vs baseline: 1.0057x; 1.0028x over previous
"""Trainium2 Bass kernel for nn_DirectedEdgeDecoder (gnn_message_passing) — v3.1.

  out[e] = W2 . relu(concat(z1[row_e], z2[col_e]) @ W1 + b1) + b2

Decomposition: per-node projections u1 = z1 @ W1[:D] + b1, u2 = z2 @ W1[D:]
(8 f16 values/node, kernel A), then per-edge combine (kernel B):
r-quantile edge sharding; u1 expanded into a DRAM slot array X by
pseudo-node (deg<=2) broadcast DMAs; u2 expanded into SBUF token streams by
in-quarter col-degree buckets {1,2}; bulk dma_scatter_add routes each token
onto its edge slot; strided readback + relu + .W2 + tree-reduce.

Scheduling (cost-model driven): DMA work balanced across the three
DMA-capable engines (SP / Act / Pool-SWDGE), ordered so quarter q's scatter
dependencies (idx, X-expansion, token stream) complete just in time; the
whole per-slot compute chain runs in f16 with DVE 2x/4x perf modes, with
tail ops placed on Pool/Act to keep DVE the only near-critical engine.
"""
import numpy as np
import concourse.bass as bass
import concourse.mybir as mybir
import concourse.tile as tile
from concourse import bacc
from concourse.bass_utils import run_bass_kernel_spmd

P = 128
N_CORES = 8
N_NODES = 100000
N_EDGES = 800000
D = 128
H = 8

f32 = mybir.dt.float32
f16 = mybir.dt.float16
i16 = mybir.dt.int16

# ------------------------------------------------------------- configuration


class Cfg:
    """Geometry of kernel B. All sizes are compile-time constants."""

    def __init__(self):
        self.NQ = 4                       # slot quarters per core
        self.D1 = 2                       # u1 pseudo-node degree
        self.PN_Q = 13311                 # u1 pseudo-node cap per quarter
        self.TRASH = 2                    # trash slots per quarter
        self.CAP2 = {1: 19968, 2: 2816}   # u2 chunk caps per quarter
        self.NCHUNK = 4                   # scatter instructions per quarter
        self.SQ = self.PN_Q * self.D1 + self.TRASH    # slots / quarter
        assert self.SQ % 256 == 0 and self.SQ < 32768
        self.S = self.NQ * self.SQ
        self.BUCKETS = [1, 2]
        self.TOK2 = {d: c * d for d, c in self.CAP2.items()}
        assert all(v % 128 == 0 for v in self.TOK2.values())
        self.NTOK_USED = sum(self.TOK2.values())      # 25600
        self.KUSED = self.NTOK_USED // 128            # 200
        self.KTOT = 200
        self.NTOKQ = self.KTOT * 128                  # 26624
        self.CHUNK = self.NTOKQ // self.NCHUNK        # 6656 tokens
        assert self.CHUNK % 128 == 0, self.CHUNK
        self.KB = {}
        kb = 0
        for d in self.BUCKETS:
            self.KB[d] = kb
            kb += self.TOK2[d] // 128
        assert kb == self.KUSED
        self.ROWS2_Q = sum(self.CAP2.values())        # U2c rows per quarter
        assert self.TRASH % self.D1 == 0
        self.ROWS1_Q = self.PN_Q + self.TRASH // self.D1
        self.XCOL = 128                   # f16 per X row (256B stride)
        self.IDXW = self.NTOKQ // 16      # idx cols per quarter (1664)
        self.KOUT = self.SQ // 128        # output cols per quarter (212)


CFG = Cfg()

# ---------------------------------------------------------------- kernel A

NC_NODES = N_NODES // N_CORES          # 12500 nodes per core
KN = 98                                # node blocks per core
NP = KN * P                            # 12544 padded nodes per core


def _new_nc():
    return bacc.Bacc(
        "TRN2", target_bir_lowering=False, debug=False, num_devices=N_CORES,
        num_swdge_queues=1,
    )


def build_precompute():
    """Per-core: u[t] = zT[t].T @ W1[t*128:(t+1)*128] (+ b1 if t == 0), f16.

    Inputs : z1T [128, NP] f32 (shard, transposed, padded), z2T likewise,
             W1 [256, 8] f32, b1 [1, 8] f32
    Output : u [2, 128, KN*8] f16 -- u[t][p, k*8+h] = node (k*128+p) proj h
    """
    nc = _new_nc()
    z1T = nc.declare_dram_parameter("z1T", [P, NP], f32, isOutput=False)
    z2T = nc.declare_dram_parameter("z2T", [P, NP], f32, isOutput=False)
    W1 = nc.declare_dram_parameter("W1", [2 * D, H], f32, isOutput=False)
    b1 = nc.declare_dram_parameter("b1", [1, H], f32, isOutput=False)
    u = nc.declare_dram_parameter("u", [2, P, KN * H], f16, isOutput=True)

    CH = 7
    CW = CH * P
    NLOAD = KN // CH                       # 14 loads per table

    # consts go first on Act; z loads start immediately on SP/Pool.
    # 28 loads: SP 10, Pool 10, Act 8 (Act also does consts + the u stores).
    seq = [0, 1, 2] * 8 + [0, 1, 0, 1]     # engine per load index

    with tile.TileContext(nc) as tc:
        with (
            tc.tile_pool(name="const", bufs=1) as const_pool,
            tc.tile_pool(name="zin", bufs=9) as zin_pool,
            tc.tile_pool(name="acc", bufs=2) as acc_pool,
            tc.tile_pool(name="psum", bufs=8, space="PSUM") as psum_pool,
        ):
            engs = [nc.sync, nc.gpsimd, nc.scalar]
            w1sb = const_pool.tile([P, 2 * H], f32)
            nc.scalar.dma_start(
                out=w1sb[:].rearrange("p (t h) -> p t h", h=H),
                in_=W1[:].rearrange("(t p) h -> p t h", p=P),
            )
            b1sb = const_pool.tile([P, H], f32)
            nc.scalar.dma_start(out=b1sb[:], in_=b1[:].to_broadcast([P, H]))

            jj = 0
            for t, zT in enumerate((z1T, z2T)):
                u_acc = acc_pool.tile([P, KN * H], f16, tag="u_acc")
                for j in range(NLOAD):
                    ztile = zin_pool.tile([P, CW], f32, tag="ztile")
                    engs[seq[jj]].dma_start(
                        out=ztile[:], in_=zT[:, j * CW:(j + 1) * CW])
                    jj += 1
                    ps = psum_pool.tile([P, CH * H], f32, tag="ps")
                    for i in range(CH):
                        nc.tensor.matmul(
                            out=ps[:, i * H:(i + 1) * H],
                            lhsT=ztile[:, i * P:(i + 1) * P],
                            rhs=w1sb[:, t * H:(t + 1) * H],
                            start=True, stop=True,
                        )
                    if t == 0:
                        nc.vector.tensor_tensor(
                            out=u_acc[:, j * CH * H:(j + 1) * CH * H],
                            in0=ps[:].rearrange("p (c h) -> p c h", h=H),
                            in1=b1sb[:].unsqueeze(1).to_broadcast([P, CH, H]),
                            op=mybir.AluOpType.add,
                        )
                    else:
                        nc.vector.tensor_copy(
                            out=u_acc[:, j * CH * H:(j + 1) * CH * H], in_=ps[:]
                        )
                nc.scalar.dma_start(out=u[t], in_=u_acc[:])
    nc.compile()
    return nc


# ---------------------------------------------------------------- kernel B


def build_scatter_kernel(cfg=CFG):
    """Per-core edge decoder via expansion + scatter-add + PE reduce.

    Inputs : U1c [NQ*ROWS1_Q, 8] f16   (pseudo-node table, quarter-major)
             U2c [NQ*ROWS2_Q, 8] f16   (per-quarter degree-bucketed table)
             idx [128, NQ*IDXW] i16    (scatter dst, 16-wrapped, replicated)
             W2 [1, 8] f32, b2 [1, 1] f32
    Output : out [128, NQ*KOUT] f16
             quarter q, slot t: hp = t // (SQ/2), t2 = t % (SQ/2),
             KH = KOUT/2 -> out[t2 // KH, q*KOUT + hp*KH + t2 % KH]
    """
    nc = _new_nc()
    R1 = cfg.NQ * cfg.ROWS1_Q
    R2 = cfg.NQ * cfg.ROWS2_Q
    U1c = nc.declare_dram_parameter("U1c", [R1, H], f16, isOutput=False)
    U2c = nc.declare_dram_parameter("U2c", [R2, H], f16, isOutput=False)
    idx = nc.declare_dram_parameter(
        "idx", [P, cfg.NQ * cfg.IDXW], i16, isOutput=False)
    W2 = nc.declare_dram_parameter("W2", [1, H], f16, isOutput=False)
    b2 = nc.declare_dram_parameter("b2", [1, 1], f16, isOutput=False)
    out = nc.declare_dram_parameter(
        "out", [P, cfg.NQ * cfg.KOUT], f16, isOutput=True)

    X = [
        nc.dram_tensor(f"Xscratch{q}", (cfg.SQ, cfg.XCOL), f16, kind="Internal")
        for q in range(cfg.NQ)
    ]

    with tile.TileContext(nc) as tc:
        with (
            tc.tile_pool(name="const", bufs=1) as const_pool,
            tc.tile_pool(name="tok", bufs=1) as tok_pool,
            tc.tile_pool(name="rb", bufs=3) as rb_pool,
        ):
            idxs = const_pool.tile([P, cfg.NQ * cfg.IDXW], i16)
            Y = [tok_pool.tile([P, cfg.KTOT * H], f16, tag=f"y{q}",
                               name=f"y{q}")
                 for q in range(cfg.NQ)]

            def load_idx(eng, q, c0=0, c1=None):
                c1 = cfg.IDXW if c1 is None else c1
                eng.dma_start(
                    out=idxs[:, q * cfg.IDXW + c0:q * cfg.IDXW + c1],
                    in_=idx[:, q * cfg.IDXW + c0:q * cfg.IDXW + c1])

            def expand_u1(eng, q):
                eng.dma_start(
                    out=X[q][:, 0:H].rearrange("(n d) h -> n d h", d=cfg.D1),
                    in_=U1c[q * cfg.ROWS1_Q:(q + 1) * cfg.ROWS1_Q, :]
                    .unsqueeze(1)
                    .to_broadcast([cfg.ROWS1_Q, cfg.D1, H]),
                )

            def expand_u2(eng, q, d):
                npp = cfg.CAP2[d] // 128
                kb = cfg.KB[d]
                kw = cfg.TOK2[d] // 128
                secbase = q * cfg.ROWS2_Q + (0 if d == 1 else cfg.CAP2[1])
                eng.dma_start(
                    out=Y[q][:, kb * H:(kb + kw) * H]
                    .rearrange("p (n d h) -> p n d h", d=d, h=H),
                    in_=U2c[secbase: secbase + cfg.CAP2[d], :]
                    .rearrange("(p n) h -> p n h", p=P)
                    .unsqueeze(2)
                    .to_broadcast([P, npp, d, H]),
                )

            # ---- phase 1, dependency-priority order ------------------
            # SP: X0 idx0 y1d1 idx2 y1d2 X3 y3d1; Act: idx1 X1 y2d1 idx3 X2
            # y2d2 y3d2 consts; Pool: y0d1 y0d2 then the scatter stream.
            ICH0 = cfg.CHUNK // 16
            load_idx(nc.sync, 0, 0, ICH0)
            load_idx(nc.scalar, 0, ICH0, None)
            expand_u1(nc.sync, 0)
            expand_u2(nc.gpsimd, 0, 1)
            expand_u2(nc.scalar, 0, 2)

            expand_u1(nc.scalar, 1)
            expand_u2(nc.sync, 1, 1)
            load_idx(nc.scalar, 1)

            load_idx(nc.sync, 2)
            expand_u2(nc.scalar, 2, 1)
            expand_u2(nc.sync, 1, 2)
            expand_u1(nc.sync, 2)

            load_idx(nc.scalar, 3)
            expand_u1(nc.sync, 3)
            expand_u2(nc.sync, 3, 1)
            expand_u2(nc.scalar, 2, 2)
            expand_u2(nc.sync, 3, 2)

            for q in range(cfg.NQ):
                if cfg.KUSED < cfg.KTOT:
                    nc.vector.memset(Y[q][:, cfg.KUSED * H:], 0.0)

            w2sb = const_pool.tile([P, H], f16)
            nc.scalar.dma_start(out=w2sb[:], in_=W2[:].to_broadcast([P, H]))
            b2sb = const_pool.tile([P, 1], f16)
            nc.scalar.dma_start(out=b2sb[:], in_=b2[:].to_broadcast([P, 1]))

            # ---- scatter-add tokens into X (Pool) --------------------
            KCH = cfg.CHUNK // 128
            ICH = cfg.CHUNK // 16
            for q in range(cfg.NQ):
                for j in range(cfg.NCHUNK):
                    nc.gpsimd.dma_scatter_add(
                        out_ap=X[q][:, 0:H],
                        in_ap=Y[q][:, j * KCH * H:(j + 1) * KCH * H]
                        .rearrange("p (k h) -> p k h", h=H),
                        idxs_ap=idxs[
                            :, q * cfg.IDXW + j * ICH:
                            q * cfg.IDXW + (j + 1) * ICH],
                        num_idxs=cfg.CHUNK,
                        num_idxs_reg=cfg.CHUNK,
                        elem_size=H,
                        elem_step=cfg.XCOL,
                        queue_num=0,
                    )

            # ---- readback + fused relu*|W2| + signed tree-reduce (+b2)
            KH = cfg.KOUT // 2                   # out cols per half piece
            rb_engs = [nc.gpsimd, nc.sync, nc.scalar, nc.sync,
                       nc.scalar, nc.sync, nc.scalar, nc.sync]
            out_acc = const_pool.tile([P, cfg.NQ * cfg.KOUT], f16)
            for q in range(cfg.NQ):
                for hp in range(2):
                    piece = q * 2 + hp
                    rb = rb_pool.tile([P, KH * H], f16, tag="rb")
                    r0 = hp * (cfg.SQ // 2)
                    rb_engs[piece].dma_start(
                        out=rb[:].rearrange("p (k h) -> p k h", h=H),
                        in_=X[q][r0:r0 + cfg.SQ // 2, 0:H]
                        .rearrange("(p k) h -> p k h", p=P),
                    )
                    # relu on DVE (4x f16), then *W2 broadcast (2x)
                    nc.vector.tensor_scalar(
                        out=rb[:], in0=rb[:], scalar1=0.0, scalar2=None,
                        op0=mybir.AluOpType.max)
                    meng = nc.vector
                    prod = rb_pool.tile([P, KH * H], f16, tag="prod")
                    meng.tensor_tensor(
                        out=prod[:].rearrange("p (k h) -> p k h", h=H),
                        in0=rb[:].rearrange("p (k h) -> p k h", h=H),
                        in1=w2sb[:].unsqueeze(1).to_broadcast([P, KH, H]),
                        op=mybir.AluOpType.mult,
                    )
                    pv = prod[:].rearrange("p (k h) -> p k h", h=H)
                    t1 = rb_pool.tile([P, KH * 4], f16, tag="t1")
                    meng.tensor_tensor(
                        out=t1[:].rearrange("p (k h) -> p k h", h=4),
                        in0=pv[:, :, 0:4], in1=pv[:, :, 4:8],
                        op=mybir.AluOpType.add,
                    )
                    t1v = t1[:].rearrange("p (k h) -> p k h", h=4)
                    t2 = rb_pool.tile([P, KH * 2], f16, tag="t2")
                    teng = nc.gpsimd
                    teng.tensor_tensor(
                        out=t2[:].rearrange("p (k h) -> p k h", h=2),
                        in0=t1v[:, :, 0:2], in1=t1v[:, :, 2:4],
                        op=mybir.AluOpType.add,
                    )
                    t2v = t2[:].rearrange("p (k h) -> p k h", h=2)
                    t3 = rb_pool.tile([P, KH], f16, tag="t3")
                    teng.tensor_tensor(
                        out=t3[:].rearrange("p (k h) -> p k h", h=1),
                        in0=t2v[:, :, 0:1], in1=t2v[:, :, 1:2],
                        op=mybir.AluOpType.add,
                    )
                    teng.tensor_tensor(
                        out=out_acc[:, q * cfg.KOUT + hp * KH:
                                    q * cfg.KOUT + (hp + 1) * KH],
                        in0=t3[:], in1=b2sb[:].to_broadcast([P, KH]),
                        op=mybir.AluOpType.add,
                    )
                if q == 1:
                    nc.scalar.dma_start(
                        out=out[:, 0:2 * cfg.KOUT],
                        in_=out_acc[:, 0:2 * cfg.KOUT])
            nc.sync.dma_start(
                out=out[:, 2 * cfg.KOUT:], in_=out_acc[:, 2 * cfg.KOUT:])
    nc.compile()
    return nc


# ------------------------------------------------------------ host planning


def plan_core(rows, cols, cfg=CFG):
    """Plan one core's slot/token layout.

    rows, cols: int arrays [E_c] of node ids (r in core's range, c global)
    for the core's edges, in original edge order.
    """
    E = len(rows)
    assert len(cols) == E

    # ---- u1 side: group edges by row node, split into deg<=D1 pseudo-nodes
    order = np.argsort(rows, kind="stable")
    srows = rows[order]
    uniq, starts = np.unique(srows, return_index=True)
    ends = np.append(starts[1:], E)

    pseudo_node = []
    pseudo_edges = []
    for n, s, e in zip(uniq, starts, ends):
        ecnt = e - s
        for off in range(0, ecnt, cfg.D1):
            pseudo_node.append(n)
            pseudo_edges.append(order[s + off: s + off + min(cfg.D1, ecnt - off)])
    NPN = len(pseudo_node)
    assert NPN <= cfg.NQ * cfg.PN_Q, (NPN, cfg.NQ * cfg.PN_Q)
    pnq = -(-NPN // cfg.NQ)                       # balanced quarter split
    assert pnq <= cfg.PN_Q

    u1_rows = np.full(cfg.NQ * cfg.ROWS1_Q, -1, dtype=np.int64)
    slot_of = np.full(E, -1, dtype=np.int64)
    for q in range(cfg.NQ):
        lo, hi = q * pnq, min((q + 1) * pnq, NPN)
        for slot_j, i in enumerate(range(lo, hi)):
            u1_rows[q * cfg.ROWS1_Q + slot_j] = pseudo_node[i]
            base = q * cfg.SQ + slot_j * cfg.D1
            for t, eid in enumerate(pseudo_edges[i]):
                slot_of[eid] = base + t
    assert (slot_of >= 0).all()

    # ---- u2 side: per quarter, bucket cols by in-quarter degree (chunks <=2)
    quarter_of = slot_of // cfg.SQ
    u2_rows = np.full(cfg.NQ * cfg.ROWS2_Q, -1, dtype=np.int64)
    idx16 = np.full((16, cfg.NQ * cfg.IDXW), 0, dtype=np.int16)
    trash_local = cfg.SQ - 1
    for q in range(cfg.NQ):
        idx16[:, q * cfg.IDXW:(q + 1) * cfg.IDXW] = trash_local

    for q in range(cfg.NQ):
        m = quarter_of == q
        qcols = cols[m]
        qeids = np.nonzero(m)[0]
        order_c = np.argsort(qcols, kind="stable")
        sc = qcols[order_c]
        uniq_c, st_c = np.unique(sc, return_index=True)
        en_c = np.append(st_c[1:], len(sc))
        chunks = {d: [] for d in cfg.BUCKETS}
        mx = cfg.BUCKETS[-1]
        for n, s, e in zip(uniq_c, st_c, en_c):
            for off in range(s, e, mx):
                grp = order_c[off: min(off + mx, e)]
                chunks[len(grp)].append((n, qeids[grp]))
        rowbase = q * cfg.ROWS2_Q
        for d in cfg.BUCKETS:
            cap = cfg.CAP2[d]
            lst = chunks[d]
            assert len(lst) <= cap, (q, d, len(lst), cap)
            npp = cap // 128
            for slot_j, (n, eids) in enumerate(lst):
                u2_rows[rowbase + slot_j] = n
                pp = slot_j // npp
                kk = cfg.KB[d] + (slot_j % npp) * d
                for rep, eid in enumerate(eids):
                    i_tok = (kk + rep) * 128 + pp
                    local = slot_of[eid] - q * cfg.SQ
                    idx16[i_tok % 16, q * cfg.IDXW + i_tok // 16] = local
            rowbase += cap

    return {
        "u1_rows": u1_rows,
        "u2_rows": u2_rows,
        "idx": idx16,
        "slot_of": slot_of,
    }


def shard_edges(edge_row):
    """r-quantile sharding: (node_hi[8], edge_core[E])."""
    counts = np.bincount(edge_row, minlength=N_NODES)
    cum = np.cumsum(counts)
    node_hi = np.zeros(N_CORES, dtype=np.int64)
    tgt = N_EDGES / N_CORES
    for c in range(N_CORES - 1):
        node_hi[c] = np.searchsorted(cum, tgt * (c + 1))
    node_hi[N_CORES - 1] = N_NODES
    edge_core = np.searchsorted(node_hi, edge_row, side="right")
    return node_hi, edge_core


# --------------------------------------------------------------- host glue


def gather_u(uA, table, nodes):
    """uA: per-core [2, 128, KN*8] f16 arrays. Returns [len(nodes), 8] f16,
    zeros where nodes == -1."""
    res = np.zeros((len(nodes), H), dtype=np.float16)
    valid = nodes >= 0
    n = nodes[valid]
    c = n // NC_NODES
    m = n % NC_NODES
    p, k = m % P, m // P
    vals = np.empty((len(n), H), dtype=np.float16)
    for cc in range(N_CORES):
        sel = c == cc
        if sel.any():
            tbl = uA[cc][table].reshape(P, KN, H)
            vals[sel] = tbl[p[sel], k[sel]]
    res[valid] = vals
    return res


def prep_precompute_inputs(z1, z2, W1, b1):
    W1 = np.ascontiguousarray(W1, dtype=np.float32)
    b1 = np.ascontiguousarray(b1, dtype=np.float32).reshape(1, H)
    in_maps = []
    for c in range(N_CORES):
        m = {}
        for name, z in (("z1T", z1), ("z2T", z2)):
            sh = np.zeros((NP, D), dtype=np.float32)
            sh[:NC_NODES] = z[c * NC_NODES:(c + 1) * NC_NODES]
            m[name] = np.ascontiguousarray(sh.T)
        m["W1"] = W1
        m["b1"] = b1
        in_maps.append(m)
    return in_maps


_CACHE = {}


def _get_kernels():
    if "a" not in _CACHE:
        _CACHE["a"] = build_precompute()
        _CACHE["b"] = build_scatter_kernel()
    return _CACHE["a"], _CACHE["b"]


LAST_IN_MAPS = {}


def kernel(z1, z2, edge_index, W1, b1, W2, b2):
    z1 = np.asarray(z1, dtype=np.float32)
    z2 = np.asarray(z2, dtype=np.float32)
    edge_index = np.asarray(edge_index)
    W2 = np.ascontiguousarray(np.asarray(W2, dtype=np.float32)
                              .reshape(1, H).astype(np.float16))
    b2 = np.ascontiguousarray(np.asarray(b2, dtype=np.float32)
                              .reshape(1, 1).astype(np.float16))
    cfg = CFG

    nc_a, nc_b = _get_kernels()
    core_ids = list(range(N_CORES))

    # ---- launch A
    in_maps_a = prep_precompute_inputs(z1, z2, W1, b1)
    LAST_IN_MAPS["a"] = in_maps_a
    res_a = run_bass_kernel_spmd(nc_a, in_maps_a, core_ids)
    uA = [res_a.results[c]["u"] for c in range(N_CORES)]

    # ---- plan + launch B
    row = np.asarray(edge_index[0], dtype=np.int64)
    col = np.asarray(edge_index[1], dtype=np.int64)
    node_hi, edge_core = shard_edges(row)

    in_maps, plans, eids = [], [], []
    for c in range(N_CORES):
        m = edge_core == c
        eid = np.nonzero(m)[0]
        plan = plan_core(row[eid], col[eid], cfg)
        U1c = gather_u(uA, 0, plan["u1_rows"])
        U2c = gather_u(uA, 1, plan["u2_rows"])
        in_maps.append({
            "U1c": np.ascontiguousarray(U1c),
            "U2c": np.ascontiguousarray(U2c),
            "idx": np.ascontiguousarray(np.tile(plan["idx"], (8, 1))),
            "W2": W2,
            "b2": b2,
        })
        plans.append(plan)
        eids.append(eid)
    LAST_IN_MAPS["b"] = in_maps
    res_b = run_bass_kernel_spmd(nc_b, in_maps, core_ids)

    # ---- unpermute (device out is f16; upcast on host)
    out = np.empty((N_EDGES, 1), dtype=np.float32)
    for c in range(N_CORES):
        o = res_b.results[c]["out"]          # [128, NQ*KOUT] f16
        vals = slot_values(o, plans[c]["slot_of"], cfg)
        out[eids[c], 0] = vals.astype(np.float32)
    return out


def slot_values(o, slot, cfg=CFG):
    """Map kernel B 'out' [128, NQ*KOUT] to per-slot values."""
    KH = cfg.KOUT // 2
    q, t = slot // cfg.SQ, slot % cfg.SQ
    hp, t2 = t // (cfg.SQ // 2), t % (cfg.SQ // 2)
    return o[t2 // KH, q * cfg.KOUT + hp * KH + t2 % KH]
